# revision 8
# baseline (speedup 1.0000x reference)
"""Trainium2 Bass kernel for nn_Interpolator (quadratic-form kernel interpolation).

Math (T=8192 targets, C=8192 contexts, D=64, DY=32):
    S = W + W^T
    scores[t,c] = (z_t - z_c)^T W (z_t - z_c)
                = q_tt[t] + q_cc[c] - z_t^T S z_c
    theta = exp(-scores);  out = (theta @ y_context) / theta.sum(-1, keepdim)

q_tt[t] scales whole theta rows and cancels in the normalization -> dropped.
q_cc[c] = 0.5 * z_c^T S z_c is folded into the main matmul contraction:
the stationary operand LC has 128 rows: rows 0..63 = zc^T, rows 64..127 =
0.5*(zc .* (S zc)); the moving operand RT has rows 0..63 = S^T z_t and
rows 64..127 = -1. A single K=128 fp16 matmul then yields cross - q_cc
directly (matmul cost depends only on moving columns, so the fold is free).

Sharding: data-parallel over targets; each of the 8 cores takes T/8 = 1024
targets and the full context set.

Per-core device program, v3 — the ACT exp stream is the roofline (8.4M
exps/core at ~1 elem/lane/cycle @ 1.2 GHz + ~150-300 cyc/instruction), so
chunks run in groups of 3 with one 1024-wide + one 2048-wide ACTIVATE per
group; everything else is engineered to never stall that stream:
  - PSUM: PB [128,1024] (2 banks) chunk 3g; PA [128,2048] (4 banks)
    chunks 3g+1/3g+2; o2 [128,1024] (2 banks) output accumulator.
  - mm2 is column-tiled: chunk pairs' [33,512] products run CONCURRENTLY
    in PE column-groups 0 (partitions 0:32+) and 2 (partitions 64:96+),
    halving mm2's PE cost; the two bands are summed on the host after the
    gather (DVE cannot add across partition bands). A-pairs (3g+1,3g+2)
    and B-pairs (B_{g-1},B_g on odd g) each take one N=512 slot per half.
  - input DMAs ride the two hardware DGE rings only (SWDGE is far too
    slow): sync ring carries W -> zc[0:512] -> zt -> zc[512:1024] ->
    zc bulk -> y h1 (ordered by first use); the scalar ring carries y h0
    and later the h0 output. Dep-free filler matmuls into the o2 banks
    keep the PE busy through the DMA phase so HAM reaches 8/8 before
    chunk 0 and the Tile scheduler slots real matmuls in as they ready.
  - the 16 zs pieces (LC rows 64..127): 0-1 in the prelude (direct DVE
    write), the rest paced one-per-group borrowing PB right after its
    exp with TWO groups of lead for the staging SBUF->SBUF DMA.
  - epilogue: chunk 63's exp is split 2x512; the final mm2 B-pair stops
    o2; DVE+ACT copy the two bands of each half in parallel and each
    half is DMA'd on its own warm ring.
Host: shard/transpose/cast inputs (layout only); gather per-core [97,1024]
outputs, sum the two bands, divide numerator rows by the denominator row.
"""

import ml_dtypes
import numpy as np

import concourse.bacc as bacc
import concourse.bass as bass
import concourse.mybir as mybir
import concourse.tile as tile
from concourse.bass_utils import run_bass_kernel_spmd

F32 = mybir.dt.float32
F16 = mybir.dt.float16
BF16 = mybir.dt.bfloat16

T, C, D, DY = 8192, 8192, 64, 32
NCORES = 8
TL = T // NCORES          # 1024 targets per core
NCHUNK = C // 128         # 64 context chunks of 128
NPIECE = C // 512         # 16 zs pieces of 512 contexts
NGROUP = 21               # full groups: B=3g, A=(3g+1, 3g+2); chunk 63 lone B
NWARM = 4
NFILL = 8
OB = 64                   # second mm2 band base partition (column-group 2)


def _build_kernel_body(tc: tile.TileContext):
    nc = tc.nc
    Exp = mybir.ActivationFunctionType.Exp

    wwt_d = nc.dram_tensor("wwt", [D, 2 * D], F32, kind="ExternalInput")
    zt_d = nc.dram_tensor("ztt", [D, TL], F16, kind="ExternalInput")
    zca_d = nc.dram_tensor("zca", [D, 512], F16, kind="ExternalInput")
    zcb_d = nc.dram_tensor("zcb", [D, 512], F16, kind="ExternalInput")
    zcv_d = nc.dram_tensor("zcv", [D, 3584], F16, kind="ExternalInput")
    zct_d = nc.dram_tensor("zct", [D, 3584], F16, kind="ExternalInput")
    y_d = nc.dram_tensor("yck", [128, NCHUNK * DY], BF16, kind="ExternalInput")
    out_d = nc.dram_tensor("out", [OB + DY + 1, TL], F32, kind="ExternalOutput")

    with (
        tc.tile_pool(name="sb", bufs=1) as sb,
        tc.tile_pool(name="pp", bufs=1, space="PSUM") as pp,
    ):
        # ---- resident SBUF slabs ----
        LC = sb.tile([128, C], F16, name="lc")
        RT = sb.tile([128, TL], F16, name="rt")
        ZT = sb.tile([D, TL], F16, name="zt")
        YT = sb.tile([128, NCHUNK * DY], BF16, name="yt")
        YA = sb.tile([128, NCHUNK, DY + 1], BF16, name="ya")
        WW = sb.tile([D, 2 * D], F32, name="ww")
        SS = sb.tile([D, D], F16, name="ss")
        SSH = sb.tile([D, D], F16, name="ssh")
        THA0 = sb.tile([128, 2048], BF16, name="tha0")
        THA1 = sb.tile([128, 2048], BF16, name="tha1")
        THB0 = sb.tile([128, TL], BF16, name="thb0")
        THB1 = sb.tile([128, TL], BF16, name="thb1")
        OSB = sb.tile([OB + DY + 1, TL], F32, name="osb")
        WRM = sb.tile([128, 512], BF16, name="wrm")
        EXD = sb.tile([D, 1], F32, name="exd")
        LCS0 = sb.tile([D, 512], F16, name="lcs0")
        LCS1 = sb.tile([D, 512], F16, name="lcs1")
        LCS = [LCS0, LCS1]
        THA = [THA0, THA1]
        THB = [THB0, THB1]

        # ---- PSUM: PA 4 banks, PB 2 banks, o2 2 banks ----
        PA = pp.tile([128, 2048], F32, tag="pa", name="pa")
        PB = pp.tile([128, TL], F32, tag="pb", name="pb")
        o2 = pp.tile([128, TL], F32, tag="o2", name="o2")

        # ---- input DMAs. scalar ring: y h0 only (lands ~9us for the
        # prelude ya piece). sync ring: everything else in first-use
        # order; the sync engine is idle so the doorbells are free. ----
        half_y = NCHUNK * DY // 2
        nc.scalar.dma_start(out=YT[:, :half_y], in_=y_d.ap()[:, :half_y])
        nc.sync.dma_start(out=WW, in_=wwt_d.ap())
        nc.sync.dma_start(out=LC[:D, 0:512], in_=zca_d.ap())
        nc.sync.dma_start(out=ZT, in_=zt_d.ap())
        nc.sync.dma_start(out=LC[:D, 512:1024], in_=zcb_d.ap())
        nc.sync.dma_start(out=LC[:D, 1024:4608], in_=zcv_d.ap())
        nc.sync.dma_start(out=LC[:D, 4608:8192], in_=zct_d.ap())
        nc.sync.dma_start(out=YT[:, half_y:], in_=y_d.ap()[:, half_y:])

        # exp-table preload (so the first chunk ACTIVATE is cheap)
        nc.vector.memset(EXD, 0.0)
        nc.scalar.activation(EXD, EXD, Exp)

        # ---- PE warm-up: 4 matmuls into PA + dep-free fillers into the
        # o2 banks; the scheduler slots real matmuls in as they ready,
        # so these keep PE busy (HAM -> 8/8) through the DMA phase. ----
        nc.vector.memset(WRM, 0.5)
        for i in range(NWARM):
            nc.tensor.matmul(
                PA[:, i * 512 : (i + 1) * 512], WRM[:, 0:128], WRM,
                start=True, stop=True,
            )

        # ---- DVE prelude chain ----
        nc.vector.memset(OSB[32:OB, :], 0.0)   # rows the out-DMA spans
        nc.vector.memset(RT[D:128, :], -1.0)
        nc.vector.tensor_add(SS, WW[:, 0:D], WW[:, D : 2 * D])   # fp16 S
        nc.vector.tensor_scalar_mul(SSH, SS, 0.5)                # fp16 S/2

        # ---- prelude zs pieces 0,1 (contexts 0:1024) staged in PA banks
        # 2-3, DVE writes LC rows 64:128 directly ----
        for k in range(2):
            sl = slice(512 * k, 512 * (k + 1))
            st = slice(1024 + 512 * k, 1024 + 512 * (k + 1))
            nc.tensor.matmul(PA[:D, st], SSH, LC[:D, sl], start=True, stop=True)
            nc.vector.tensor_mul(LC[D:128, sl], PA[:D, st], LC[:D, sl])

        # ---- RT rows 0..63 = S^T zt, staged in PA banks 0-1; cast h0 on
        # ACT (idle) and h1 on DVE in parallel ----
        for h in range(2):
            sl = slice(h * 512, (h + 1) * 512)
            nc.tensor.matmul(PA[:D, sl], SS, ZT[:, sl], start=True, stop=True)
        nc.scalar.copy(RT[:D, 0:512], PA[:D, 0:512])
        nc.vector.tensor_copy(RT[:D, 512:1024], PA[:D, 512:1024])

        # y_aug piece q (16 chunks): [128, 16, 33]; col 32 = 1.0
        nc.vector.memset(YA[:, :, DY : DY + 1], 1.0)
        qy = NCHUNK // 4 * DY

        def ya_piece(q, eng):
            eng(
                YA[:, q * 16 : (q + 1) * 16, 0:DY],
                YT[:, q * qy : (q + 1) * qy].rearrange("p (j d) -> p j d", d=DY),
            )

        ya_piece(0, nc.scalar.copy)   # ACT is idle in the prelude

        # PE fillers into the o2 banks (mm2's start=True clears them)
        for i in range(NFILL):
            nc.tensor.matmul(
                o2[:, (i % 2) * 512 : (i % 2 + 1) * 512], WRM[:, 0:128], WRM,
                start=True, stop=True,
            )

        # in-loop zs piece: borrow PB[:D, 0:512] right after exp(B_g);
        # the DVE mul lands in SBUF staging and an SBUF->SBUF DMA
        # (subtile-tracked) carries it into LC rows 64..127 so loop
        # matmuls wait only on the DMA. Two groups of lead.
        def zs_piece(k):
            sl = slice(512 * k, 512 * (k + 1))
            nc.tensor.matmul(PB[:D, 0:512], SSH, LC[:D, sl], start=True,
                             stop=True)
            nc.vector.tensor_mul(LCS[k % 2][:, 0:512], PB[:D, 0:512],
                                 LC[:D, sl])
            nc.sync.dma_start(out=LC[D:128, sl], in_=LCS[k % 2][:, 0:512])

        # piece k emitted 2 groups before its first consumer chunk 4k
        zs_at = {}
        for k in range(2, NPIECE):
            c = 4 * k
            first_use_iter = c // 3 - 1 if c % 3 == 0 else (c - 1) // 3
            g_k = max(0, first_use_iter - 2)
            while g_k in zs_at:
                g_k += 1
            zs_at[g_k] = k

        def score_mms(P, base, c):
            """two N=512 matmuls: scores of chunk c into P[:, base:base+1024]"""
            lhsT = LC[:, c * 128 : (c + 1) * 128]
            for h in range(2):
                nc.tensor.matmul(
                    P[:, base + h * 512 : base + (h + 1) * 512],
                    lhsT,
                    RT[:, h * 512 : (h + 1) * 512],
                    start=True, stop=True,
                )

        started = {}

        def mm2_pair(th1, b1, c1, th2, b2, c2, stop=False):
            """column-tiled mm2: chunk c1 -> band 0, chunk c2 -> band OB,
            concurrently; one call covers both target halves. Each
            (half, band) accumulation group starts on its first write —
            start=True clears has_written only for that band's partition
            rows, so the bands are independent."""
            for h in range(2):
                sl = slice(h * 512, (h + 1) * 512)
                for band, th, b, c in ((0, th1, b1, c1), (OB, th2, b2, c2)):
                    nc.tensor.matmul(
                        o2[band : band + DY + 1, sl],
                        YA[:, c, :],
                        th[:, b + h * 512 : b + (h + 1) * 512],
                        start=not started.get((h, band), False), stop=stop,
                        tile_position=(0, band),
                        # the sim's group-started map aliases the two
                        # bands of one bank; its per-row pending-zero
                        # value model is still exact
                        skip_group_check=True,
                    )
                    started[(h, band)] = True

        # ---- prologue of the software pipeline ----
        score_mms(PB, 0, 0)                      # B(0)

        # ---- main loop over 21 full groups ----
        for g in range(NGROUP):
            cB, cA1, cA2 = 3 * g, 3 * g + 1, 3 * g + 2
            nc.scalar.activation(THB[g % 2], PB, Exp)          # exp(B_g)
            score_mms(PA, 0, cA1)                              # A(g)
            score_mms(PA, 1024, cA2)
            nc.scalar.activation(THA[g % 2], PA, Exp)          # exp(A_g)
            if g in zs_at:
                zs_piece(zs_at[g])
            score_mms(PB, 0, cB + 3)                           # B(g+1)
            if g > 0:
                tha = THA[(g - 1) % 2]
                mm2_pair(tha, 0, cA1 - 3, tha, 1024, cA2 - 3)  # mm2(A_{g-1})
            if g % 2 == 1:
                mm2_pair(THB[(g - 1) % 2], 0, cB - 3,
                         THB[g % 2], 0, cB)                    # mm2(B-pair)
            if g in (3, 8, 13):
                ya_piece({3: 1, 8: 2, 13: 3}[g], nc.vector.tensor_copy)

        # ---- epilogue: final chunk 63 in PB, exp split 2x512; close the
        # accumulators; band copies on DVE+ACT; one out-DMA per ring ----
        g = NGROUP
        for h in range(2):
            sl = slice(h * 512, (h + 1) * 512)
            nc.scalar.activation(THB[g % 2][:, sl], PB[:, sl], Exp)
        tha = THA[(g - 1) % 2]
        mm2_pair(tha, 0, 61, tha, 1024, 62)                    # mm2(A_20)
        mm2_pair(THB[(g - 1) % 2], 0, 60,
                 THB[g % 2], 0, 63, stop=True)                 # mm2(B20,B21)
        nc.vector.tensor_copy(OSB[0 : DY + 1, 0:512], o2[0 : DY + 1, 0:512])
        nc.scalar.copy(OSB[OB : OB + DY + 1, 0:512], o2[OB : OB + DY + 1, 0:512])
        nc.scalar.dma_start(out=out_d.ap()[:, 0:512], in_=OSB[:, 0:512])
        nc.vector.tensor_copy(OSB[0 : DY + 1, 512:1024], o2[0 : DY + 1, 512:1024])
        nc.scalar.copy(OSB[OB : OB + DY + 1, 512:1024],
                       o2[OB : OB + DY + 1, 512:1024])
        nc.sync.dma_start(out=out_d.ap()[:, 512:1024], in_=OSB[:, 512:1024])


_CACHED = None


def _get_nc():
    global _CACHED
    if _CACHED is None:
        nc = bacc.Bacc(
            "TRN2",
            target_bir_lowering=False,
            debug=False,
            enable_asserts=False,
        )
        with tile.TileContext(nc) as tc:
            _build_kernel_body(tc)
        nc.compile()
        _CACHED = nc
    return _CACHED


def make_in_maps(z_context, y_context, z_target, W):
    """Host-side layout prep (transpose/reshape/cast only) + sharding."""
    z_context = np.asarray(z_context, dtype=np.float32)
    y_context = np.asarray(y_context, dtype=np.float32)
    z_target = np.asarray(z_target, dtype=np.float32)
    W = np.asarray(W, dtype=np.float32)

    zcT = np.ascontiguousarray(z_context.T.astype(np.float16))  # [64, 8192]
    zca = np.ascontiguousarray(zcT[:, 0:512])
    zcb = np.ascontiguousarray(zcT[:, 512:1024])
    zcv = np.ascontiguousarray(zcT[:, 1024:4608])
    zct = np.ascontiguousarray(zcT[:, 4608:8192])
    # chunk j partition p holds context j*128+p:
    # yck[p, j*DY+d] = y_context[j*128+p, d]
    yck = np.ascontiguousarray(
        y_context.reshape(NCHUNK, 128, DY).transpose(1, 0, 2).reshape(
            128, NCHUNK * DY
        )
    ).astype(ml_dtypes.bfloat16)
    wwt = np.ascontiguousarray(np.concatenate([W, W.T], axis=1))  # [64, 128]

    in_maps = []
    for i in range(NCORES):
        ztT = np.ascontiguousarray(
            z_target[i * TL : (i + 1) * TL].T.astype(np.float16)
        )
        m = {
            "wwt": wwt, "ztt": ztT, "yck": yck,
            "zca": zca, "zcb": zcb, "zcv": zcv, "zct": zct,
        }
        in_maps.append(m)
    return in_maps


def postprocess(results):
    """Gather per-core [97, TL] band outputs -> full (T, DY) output."""
    outs = []
    for r in results:
        o = r["out"]
        merged = (o[0 : DY + 1] + o[OB : OB + DY + 1]).T  # [TL, 33]
        outs.append(merged[:, :DY] / merged[:, DY : DY + 1])
    return np.concatenate(outs, axis=0).astype(np.float32)


def run(in_maps, **kwargs):
    nc = _get_nc()
    return run_bass_kernel_spmd(nc, in_maps, core_ids=list(range(NCORES)), **kwargs)


def kernel(z_context, y_context, z_target, W):
    in_maps = make_in_maps(z_context, y_context, z_target, W)
    res = run(in_maps)
    return postprocess(res.results)


# revision 9
# speedup vs baseline: 1.0264x; 1.0264x over previous
"""Trainium2 Bass kernel for nn_Interpolator (quadratic-form kernel interpolation).

Math (T=8192 targets, C=8192 contexts, D=64, DY=32):
    S = W + W^T
    scores[t,c] = (z_t - z_c)^T W (z_t - z_c)
                = q_tt[t] + q_cc[c] - z_t^T S z_c
    theta = exp(-scores);  out = (theta @ y_context) / theta.sum(-1, keepdim)

q_tt[t] scales whole theta rows and cancels in the normalization -> dropped.
q_cc[c] = 0.5 * z_c^T S z_c is folded into the main matmul contraction:
the stationary operand LC has 128 rows: rows 0..63 = zc^T, rows 64..127 =
0.5*(zc .* (S zc)); the moving operand RT has rows 0..63 = S^T z_t and
rows 64..127 = -1. A single K=128 fp16 matmul then yields cross - q_cc
directly (matmul cost depends only on moving columns, so the fold is free).

Sharding: data-parallel over targets; each of the 8 cores takes T/8 = 1024
targets and the full context set.

Per-core device program, v4. The ACT exp stream is the roofline: measured,
back-to-back 1024-wide ACTIVATEs run at ~(1024+150)/1.2 = 996 ns each
(the pipe-fill overlaps), which beats any wider-instruction scheme that
PSUM bank budget allows. So the loop is a 3-tile rotation of [128,1024]
score tiles (2 banks each) whose exps run back-to-back, and everything
else is engineered off that critical path:
  - mm2 is column-tiled: chunk PAIRS' [33,512] products run CONCURRENTLY
    in PE column-groups 0 (partitions 0:33) and 2 (partitions 64:97) of
    the o2 accumulator (2 banks), halving mm2's PE cost; the two bands
    are summed on the host after the gather (no cross-partition add on
    device). Pair (j-2, j-1) is emitted after exp(j) - both theta tiles
    are ready, and the rotation keeps WAR slack at 2 windows.
  - the 3-tile rotation gives zs pieces (LC rows 64..127) the baseline's
    safe borrow: piece at chunk j=3m+1 borrows tile (j+2)%3's region
    [512:1024] right after its exp; the DVE mul lands in SBUF staging
    and a subtile-tracked SBUF->SBUF DMA carries it into LC.
  - input DMAs ride the two hardware DGE rings only: scalar ring W then
    y h0 (the W -> S -> RT/zs chain gates chunk 0); sync ring zt ->
    zc[0:512] -> zc[512:1024] -> zc bulk -> y h1, in first-use order.
  - dep-free filler matmuls into the o2 banks keep PE busy through the
    DMA phase (HAM 8/8 before chunk 0); mm2's start=True clears them.
  - output: [97, 512] per target-half, each a CONTIGUOUS DRAM tensor on
    its own ring (a strided DRAM side serializes the whole transfer onto
    one DMA engine at ~83ns/2KB - measured), with DVE/ACT copying the
    two bands of each half in parallel; chunk 63's exp is split 2x512 so
    the final mm2 pair and the h0 path start half a window early.
Host: shard/transpose/cast inputs (layout only); gather per-core halves,
sum the two bands, divide numerator rows by the denominator row.
"""

import ml_dtypes
import numpy as np

import concourse.bacc as bacc
import concourse.bass as bass
import concourse.mybir as mybir
import concourse.tile as tile
from concourse.bass_utils import run_bass_kernel_spmd

F32 = mybir.dt.float32
F16 = mybir.dt.float16
BF16 = mybir.dt.bfloat16

T, C, D, DY = 8192, 8192, 64, 32
NCORES = 8
TL = T // NCORES          # 1024 targets per core
NCHUNK = C // 128         # 64 context chunks of 128
NPIECE = C // 512         # 16 zs pieces of 512 contexts
NWARM = 4
NFILL = 8
OB = 64                   # second mm2 band base partition (column-group 2)


def _build_kernel_body(tc: tile.TileContext):
    nc = tc.nc
    Exp = mybir.ActivationFunctionType.Exp

    wwt_d = nc.dram_tensor("wwt", [D, 2 * D], F32, kind="ExternalInput")
    zt_d = nc.dram_tensor("ztt", [D, TL], F16, kind="ExternalInput")
    zca_d = nc.dram_tensor("zca", [D, 512], F16, kind="ExternalInput")
    zcb_d = nc.dram_tensor("zcb", [D, 512], F16, kind="ExternalInput")
    zcv_d = nc.dram_tensor("zcv", [D, 3584], F16, kind="ExternalInput")
    zct_d = nc.dram_tensor("zct", [D, 3584], F16, kind="ExternalInput")
    y_d = nc.dram_tensor("yck", [128, NCHUNK * DY], BF16, kind="ExternalInput")
    out0_d = nc.dram_tensor("out0", [OB + DY + 1, 512], F32,
                            kind="ExternalOutput")
    out1_d = nc.dram_tensor("out1", [OB + DY + 1, 512], F32,
                            kind="ExternalOutput")

    with (
        tc.tile_pool(name="sb", bufs=1) as sb,
        tc.tile_pool(name="pp", bufs=1, space="PSUM") as pp,
    ):
        # ---- resident SBUF slabs ----
        LC = sb.tile([128, C], F16, name="lc")
        RT = sb.tile([128, TL], F16, name="rt")
        ZT = sb.tile([D, TL], F16, name="zt")
        YT = sb.tile([128, NCHUNK * DY], BF16, name="yt")
        YA = sb.tile([128, NCHUNK, DY + 1], BF16, name="ya")
        WW = sb.tile([D, 2 * D], F32, name="ww")
        SS = sb.tile([D, D], F16, name="ss")
        SSH = sb.tile([D, D], F16, name="ssh")
        TH0 = sb.tile([128, TL], BF16, name="th0")
        TH1 = sb.tile([128, TL], BF16, name="th1")
        TH2 = sb.tile([128, TL], BF16, name="th2")
        OSB = sb.tile([OB + DY + 1, TL], F32, name="osb")
        WRM = sb.tile([128, 512], BF16, name="wrm")
        EXD = sb.tile([D, 1], F32, name="exd")
        LCS0 = sb.tile([D, 512], F16, name="lcs0")
        LCS1 = sb.tile([D, 512], F16, name="lcs1")
        LCS = [LCS0, LCS1]
        THS = [TH0, TH1, TH2]

        # ---- PSUM: 3 rotating score tiles (6 banks) + o2 (2 banks) ----
        PS0 = pp.tile([128, TL], F32, tag="ring0", name="ps0")
        PS1 = pp.tile([128, TL], F32, tag="ring1", name="ps1")
        PS2 = pp.tile([128, TL], F32, tag="ring2", name="ps2")
        o2 = pp.tile([128, TL], F32, tag="o2", name="o2")
        PSC = [PS0, PS1, PS2]

        # ---- input DMAs on the two HWDGE rings, first-use order ----
        half_y = NCHUNK * DY // 2
        nc.scalar.dma_start(out=WW, in_=wwt_d.ap())
        nc.scalar.dma_start(out=YT[:, :half_y], in_=y_d.ap()[:, :half_y])
        nc.sync.dma_start(out=ZT, in_=zt_d.ap())
        nc.sync.dma_start(out=LC[:D, 0:512], in_=zca_d.ap())
        nc.sync.dma_start(out=LC[:D, 512:1024], in_=zcb_d.ap())
        nc.sync.dma_start(out=LC[:D, 1024:4608], in_=zcv_d.ap())
        nc.sync.dma_start(out=LC[:D, 4608:8192], in_=zct_d.ap())
        nc.sync.dma_start(out=YT[:, half_y:], in_=y_d.ap()[:, half_y:])

        # exp-table preload (so the first chunk ACTIVATE is cheap)
        nc.vector.memset(EXD, 0.0)
        nc.scalar.activation(EXD, EXD, Exp)

        # ---- PE warm-up + fillers: keep PE busy through the DMA phase
        # (HAM -> 8/8); the scheduler slots real matmuls in as ready ----
        nc.vector.memset(WRM, 0.5)
        for i in range(NWARM):
            nc.tensor.matmul(
                PSC[i % 2][:, (i // 2) * 512 : (i // 2 + 1) * 512],
                WRM[:, 0:128], WRM, start=True, stop=True,
            )

        # ---- DVE prelude chain ----
        nc.vector.memset(OSB[32:OB, :], 0.0)   # rows the out-DMA spans
        nc.vector.memset(RT[D:128, :], -1.0)
        nc.vector.tensor_add(SS, WW[:, 0:D], WW[:, D : 2 * D])   # fp16 S
        nc.vector.tensor_scalar_mul(SSH, SS, 0.5)                # fp16 S/2

        # ---- RT rows 0..63 = S^T zt staged in PS2, one DVE cast; chunk
        # 0/1 never wait on PS2 ----
        for h in range(2):
            sl = slice(h * 512, (h + 1) * 512)
            nc.tensor.matmul(PS2[:D, sl], SS, ZT[:, sl], start=True, stop=True)
        nc.vector.tensor_copy(RT[:D, 0:1024], PS2[:D, 0:1024])

        # ---- prelude zs pieces 0,1 (contexts 0:1024) staged in PS0/PS1
        # region [512:1024]; DVE writes LC rows 64:128 directly ----
        for k in range(2):
            sl = slice(512 * k, 512 * (k + 1))
            nc.tensor.matmul(PSC[k][:D, 512:1024], SSH, LC[:D, sl],
                             start=True, stop=True)
            nc.vector.tensor_mul(LC[D:128, sl], PSC[k][:D, 512:1024],
                                 LC[:D, sl])

        # y_aug piece q (16 chunks): [128, 16, 33]; col 32 = 1.0
        nc.vector.memset(YA[:, :, DY : DY + 1], 1.0)
        qy = NCHUNK // 4 * DY

        def ya_piece(q, eng):
            eng(
                YA[:, q * 16 : (q + 1) * 16, 0:DY],
                YT[:, q * qy : (q + 1) * qy].rearrange("p (j d) -> p j d", d=DY),
            )

        ya_piece(0, nc.scalar.copy)   # ACT is idle in the prelude

        # PE fillers into the o2 banks (mm2's start=True clears them)
        for i in range(NFILL):
            nc.tensor.matmul(
                o2[:, (i % 2) * 512 : (i % 2 + 1) * 512], WRM[:, 0:128], WRM,
                start=True, stop=True,
            )

        # in-loop zs piece k at chunk j=3m+1: borrow tile (j+2)%3 region
        # [512:1024] right after its exp; stage the mul in SBUF and land
        # it in LC rows 64:128 via a subtile-tracked SBUF->SBUF DMA.
        def zs_piece(k, tl):
            sl = slice(512 * k, 512 * (k + 1))
            nc.tensor.matmul(PSC[tl][:D, 512:1024], SSH, LC[:D, sl],
                             start=True, stop=True)
            nc.vector.tensor_mul(LCS[k % 2][:, 0:512], PSC[tl][:D, 512:1024],
                                 LC[:D, sl])
            nc.sync.dma_start(out=LC[D:128, sl], in_=LCS[k % 2][:, 0:512])

        def score_mms(P, c):
            lhsT = LC[:, c * 128 : (c + 1) * 128]
            for h in range(2):
                nc.tensor.matmul(
                    P[:, h * 512 : (h + 1) * 512],
                    lhsT,
                    RT[:, h * 512 : (h + 1) * 512],
                    start=True, stop=True,
                )

        started = {}

        def mm2_pair(c1, c2, stop=False):
            """column-tiled mm2: chunk c1 -> band 0, chunk c2 -> band OB,
            running concurrently; both target halves. Each (half, band)
            group starts on its first write (start=True clears only that
            band's partition rows of the bank)."""
            for h in range(2):
                sl = slice(h * 512, (h + 1) * 512)
                for band, c in ((0, c1), (OB, c2)):
                    nc.tensor.matmul(
                        o2[band : band + DY + 1, sl],
                        YA[:, c, :],
                        THS[c % 3][:, sl],
                        start=not started.get((h, band), False), stop=stop,
                        tile_position=(0, band),
                        # the sim's group-started map aliases the two
                        # bands of one bank; its per-row pending-zero
                        # value model is still exact
                        skip_group_check=True,
                    )
                    started[(h, band)] = True

        # ---- main loop over 64 context chunks ----
        for j in range(NCHUNK):
            P = PSC[j % 3]
            score_mms(P, j)
            if j % 3 == 1 and 2 + (j - 1) // 3 < NPIECE:
                zs_piece(2 + (j - 1) // 3, (j + 2) % 3)
            if j == NCHUNK - 1:
                # split the last exp so the final mm2 pair and the h0
                # evacuation start half a window earlier
                for h in range(2):
                    sl = slice(h * 512, (h + 1) * 512)
                    nc.scalar.activation(THS[j % 3][:, sl], P[:, sl], Exp)
            else:
                nc.scalar.activation(THS[j % 3], P, Exp)
            if j >= 2 and j % 2 == 0:
                mm2_pair(j - 2, j - 1)
            if j in (12, 28, 44):
                ya_piece({12: 1, 28: 2, 44: 3}[j], nc.vector.tensor_copy)

        # ---- epilogue: final pair closes o2; band copies on DVE+ACT in
        # parallel; each half on its own ring as a contiguous tensor ----
        mm2_pair(62, 63, stop=True)
        nc.vector.tensor_copy(OSB[0 : DY + 1, 0:512], o2[0 : DY + 1, 0:512])
        nc.scalar.copy(OSB[OB : OB + DY + 1, 0:512], o2[OB : OB + DY + 1, 0:512])
        nc.scalar.dma_start(out=out0_d.ap(), in_=OSB[:, 0:512])
        nc.vector.tensor_copy(OSB[0 : DY + 1, 512:1024], o2[0 : DY + 1, 512:1024])
        nc.scalar.copy(OSB[OB : OB + DY + 1, 512:1024],
                       o2[OB : OB + DY + 1, 512:1024])
        nc.sync.dma_start(out=out1_d.ap(), in_=OSB[:, 512:1024])


_CACHED = None


def _get_nc():
    global _CACHED
    if _CACHED is None:
        nc = bacc.Bacc(
            "TRN2",
            target_bir_lowering=False,
            debug=False,
            enable_asserts=False,
        )
        with tile.TileContext(nc) as tc:
            _build_kernel_body(tc)
        nc.compile()
        _CACHED = nc
    return _CACHED


def make_in_maps(z_context, y_context, z_target, W):
    """Host-side layout prep (transpose/reshape/cast only) + sharding."""
    z_context = np.asarray(z_context, dtype=np.float32)
    y_context = np.asarray(y_context, dtype=np.float32)
    z_target = np.asarray(z_target, dtype=np.float32)
    W = np.asarray(W, dtype=np.float32)

    zcT = np.ascontiguousarray(z_context.T.astype(np.float16))  # [64, 8192]
    zca = np.ascontiguousarray(zcT[:, 0:512])
    zcb = np.ascontiguousarray(zcT[:, 512:1024])
    zcv = np.ascontiguousarray(zcT[:, 1024:4608])
    zct = np.ascontiguousarray(zcT[:, 4608:8192])
    # chunk j partition p holds context j*128+p:
    # yck[p, j*DY+d] = y_context[j*128+p, d]
    yck = np.ascontiguousarray(
        y_context.reshape(NCHUNK, 128, DY).transpose(1, 0, 2).reshape(
            128, NCHUNK * DY
        )
    ).astype(ml_dtypes.bfloat16)
    wwt = np.ascontiguousarray(np.concatenate([W, W.T], axis=1))  # [64, 128]

    in_maps = []
    for i in range(NCORES):
        ztT = np.ascontiguousarray(
            z_target[i * TL : (i + 1) * TL].T.astype(np.float16)
        )
        m = {
            "wwt": wwt, "ztt": ztT, "yck": yck,
            "zca": zca, "zcb": zcb, "zcv": zcv, "zct": zct,
        }
        in_maps.append(m)
    return in_maps


def postprocess(results):
    """Gather per-core band outputs -> full (T, DY) normalized output."""
    outs = []
    for r in results:
        o = np.concatenate([r["out0"], r["out1"]], axis=1)  # [97, TL]
        merged = (o[0 : DY + 1] + o[OB : OB + DY + 1]).T    # [TL, 33]
        outs.append(merged[:, :DY] / merged[:, DY : DY + 1])
    return np.concatenate(outs, axis=0).astype(np.float32)


def run(in_maps, **kwargs):
    nc = _get_nc()
    return run_bass_kernel_spmd(nc, in_maps, core_ids=list(range(NCORES)), **kwargs)


def kernel(z_context, y_context, z_target, W):
    in_maps = make_in_maps(z_context, y_context, z_target, W)
    res = run(in_maps)
    return postprocess(res.results)
